# revision 25
# baseline (speedup 1.0000x reference)
"""Fused ControlTransformerBlock kernel for 8 Trainium2 NeuronCores.

Sharding: DP over batch (2 groups of 4 cores) x TP over heads/mlp within each
group.  Activations are kept feature-major ("xT" = [D, tokens]) so every
matmul takes natural-layout weights as lhsT.  Attention uses the S^T = [k, q]
score layout: the softmax denominator comes from a fused ones-column in V and
the rmsnorm factors are folded into the exp scale (k side) and into Q (q side).

Collectives per DP group of 4: AG1 (attn head outputs, bf16) -> col-sharded
out-proj -> AG2 (gated attn delta D1, bf16) -> TP FFN -> ReduceScatter (fp32
g2*partial) -> AG3 (img_h2^T band, fp32) -> col-sharded after_proj.  The
modulation vectors come from fp32r matmuls on 8-way interleaved column shards
plus an 8-core AllGather.
"""
import os
import sys

for _p in ("/opt/trn_rl_repo", "/root/.axon_site/_ro/trn_rl_repo"):
    if os.path.isdir(_p) and _p not in sys.path:
        sys.path.insert(0, _p)

import numpy as np
import ml_dtypes

import concourse.bass as bass
import concourse.mybir as mybir
from concourse import bacc
from concourse.tile import TileContext
from concourse.bass import ts, ds

dt = mybir.dt
AF = mybir.ActivationFunctionType
ALU = mybir.AluOpType
AX = mybir.AxisListType

if os.environ.get("BK_SMALL"):
    B, SI, ST, D, H, HD, MLP = 2, 512, 128, 512, 8, 64, 1024
else:
    B, SI, ST, D, H, HD, MLP = 2, 2048, 512, 1536, 24, 64, 6144

S = SI + ST
NCORES, TP = 8, 4
RG_ = ((H // 4) + 3) // 4
HPC = H // TP            # heads per core
DSH = D // TP            # attn col shard (= HPC*HD)
MSH = MLP // TP          # mlp shard per core
KT = D // 128            # k-tiles over D
QT = DSH // 128          # m-tiles over DSH
MT = MSH // 128          # m-tiles over MSH
TT = S // 128            # token tiles
ITT = SI // 128          # img token tiles
MODB = 6 * D // 128      # 128-blocks in the 6D mod output
JB = MODB // 8           # mod blocks per core
CHUNKS = [(i, min(512, S - i)) for i in range(0, S, 512)]
ICHUNKS = [(i, sz) for (i, sz) in CHUNKS if i < SI]
assert SI % 512 == 0

GROUPS = [[0, 1, 2, 3], [4, 5, 6, 7]]
ALL8 = [[0, 1, 2, 3, 4, 5, 6, 7]]

# mod component order from jnp.split: sh1, sc1, g1, sh2, sc2, g2
C_SH1, C_SC1, C_G1, C_SH2, C_SC2, C_G2 = range(6)

SIMCOMPOSE = bool(os.environ.get("BK_SIMCOMPOSE"))
DEBUG = bool(os.environ.get("BK_DEBUG"))

f32, f32r, bf16 = dt.float32, dt.float32r, dt.bfloat16


def _kp(ap):
    """[K, M] dram view -> [p, kt, M]"""
    return ap.rearrange("(kt p) m -> p kt m", p=128)


def _r32(ap):
    return ap.bitcast(f32r)


def build():
    nc = bacc.Bacc("TRN2", target_bir_lowering=False, num_devices=NCORES)

    def inp(name, shape, dtype=f32):
        return nc.declare_dram_parameter(name, list(shape), dtype, isOutput=False)

    def outp(name, shape, dtype=f32):
        return nc.declare_dram_parameter(name, list(shape), dtype, isOutput=True)

    x = inp("x", [S, D])                       # concat(img, txt) for this batch
    xband = inp("xband", [S, DSH])             # x[:, DSH*t : DSH*(t+1)]
    tembT = inp("tembT", [D, 2])               # both batches, feature-major
    modw = inp("modw", [2, D, 128 * JB], f32r)       # interleaved col shard, 2 streams
    modb = inp("modb", [2, JB, 128])
    bsel = inp("bsel", [128, 2])               # one-hot row for own batch
    tselb = inp("tselb", [128, QT, KT])        # one-hot band d-tile selector
    wqkv = inp("wqkv", [D, 6 * DSH])           # [wq|wk|wv|awq|awk|awv] col shards
    bqkv = inp("bqkv", [6 * DSH])
    nw = inp("nw", [DSH, 4])                   # nq,nk,naq,nak tiled per head
    hmask = inp("hmask", [DSH, HPC], bf16)     # head indicator (msq matmul rhs)
    wout = inp("wout", [2, D, DSH])            # wo/awo col shards
    bout = inp("bout", [2, DSH])
    w1 = inp("w1", [2, D, MSH])
    b1 = inp("b1", [2, MSH])
    w2 = inp("w2", [2, MSH, D])
    b2q = inp("b2q", [2, D])                   # b2 / TP
    apw = inp("apw", [D, DSH], f32r)                 # ap_w col shard
    apb = inp("apb", [DSH])
    id128f = inp("id128f", [128, 128])
    id128b = inp("id128b", [128, 128], bf16)
    bvb = inp("bvb", [2, 128, DSH])            # bv/abv broadcast to 128 rows

    if DEBUG:
        d_mod = outp("d_mod", [128, 2, 6 * KT])
        d_xt = outp("d_xt", [128, KT, S], bf16)
        d_xmod = outp("d_xmod", [128, KT, S], bf16)
        d_qt = outp("d_qt", [128, QT, S], bf16)
        d_kt = outp("d_kt", [128, QT, S], bf16)
        d_scalek = outp("d_scalek", [128, TT, HPC])
        d_rqrows = outp("d_rqrows", [128, RG_, S], bf16)
        d_vaug = outp("d_vaug", [128, TT, HPC * (HD + 1)], bf16)
        d_e = outp("d_e", [128, TT, 512], bf16)
        d_attn = outp("d_attn", [128, QT, S], bf16)
        d_d1 = outp("d_d1", [128, QT, S])
        d_ff = outp("d_ff", [128, KT, S], bf16)
        d_hb = outp("d_hb", [128, MT, S], bf16)
        d_rs = outp("d_rs", [128, QT, S])
    o_hid = outp("o_hid", [S, DSH])            # col band of concat(img_h, txt_h)
    o_ctrl = outp("o_ctrl", [SI, DSH])         # col band of control_out

    xt_dram = nc.dram_tensor("xt_dram", [KT, 128, S], bf16)
    modag_in = nc.dram_tensor("modag_in", [2 * JB * 128 * 2], f32)
    modag_out = nc.dram_tensor("modag_out", [NCORES * 2 * JB * 128 * 2], f32,
                               addr_space="Shared")
    ag1_in = nc.dram_tensor("ag1_in", [QT, 128, S], bf16)
    ag1_out = nc.dram_tensor("ag1_out", [TP * QT, 128, S], bf16)
    ag2_in = nc.dram_tensor("ag2_in", [QT, 128, S], bf16)
    ag2_out = nc.dram_tensor("ag2_out", [TP * QT, 128, S], bf16)
    rs_in = nc.dram_tensor("rs_in", [KT, 128, S], f32)
    rs_out = nc.dram_tensor("rs_out", [QT, 128, S], f32)
    ag3_in = nc.dram_tensor("ag3_in", [QT, 128, SI], f32r)
    ag3_out = nc.dram_tensor("ag3_out", [TP * QT, 128, SI], f32r)

    with TileContext(nc) as tc:
        const = tc.alloc_tile_pool(name="const", bufs=1)
        psp = tc.alloc_tile_pool(name="ps", bufs=1, space="PSUM")

        def ps_mm(w=512):
            return psp.tile([128, w], f32, name="psmm", tag="mm", bufs=3)

        def ps_pv():
            return psp.tile([HD + 1, 512], f32, name="pspv", tag="pv", bufs=2)

        def ps_aux(p=128, w=128, dtype=f32):
            return psp.tile([p, w], dtype, name="psaux", tag="aux", bufs=2)

        # ---------------- constants ----------------
        idf = const.tile([128, 128], f32)
        nc.sync.dma_start(out=idf[:], in_=id128f[:])
        idb = const.tile([128, 128], bf16)
        nc.sync.dma_start(out=idb[:], in_=id128b[:])
        bsel_sb = const.tile([128, 2], f32)
        nc.sync.dma_start(out=bsel_sb[:], in_=bsel[:])
        tselb_sb = const.tile([128, QT, KT], f32)
        nc.sync.dma_start(out=tselb_sb[:], in_=tselb[:])
        nw_sb = const.tile([128, QT, 4], f32)
        nc.sync.dma_start(out=nw_sb[:], in_=nw.rearrange("(q p) w -> p q w", p=128))
        hmask_sb = const.tile([128, QT, HPC], bf16)
        nc.sync.dma_start(out=hmask_sb[:], in_=hmask.rearrange("(q p) h -> p q h", p=128))
        bqkv_sb = const.tile([128, 6 * QT], f32)
        nc.sync.dma_start(out=bqkv_sb[:], in_=bqkv.rearrange("(m p) -> p m", p=128))
        bout_sb = const.tile([128, 2, QT], f32)
        nc.sync.dma_start(out=bout_sb[:], in_=bout.rearrange("s (q p) -> p s q", p=128))
        b1_sb = const.tile([128, 2, MT], f32)
        nc.sync.dma_start(out=b1_sb[:], in_=b1.rearrange("s (m p) -> p s m", p=128))
        b2q_sb = const.tile([128, 2, KT], f32)
        nc.sync.dma_start(out=b2q_sb[:], in_=b2q.rearrange("s (k p) -> p s k", p=128))
        apb_sb = const.tile([128, QT], f32)
        nc.sync.dma_start(out=apb_sb[:], in_=apb.rearrange("(q p) -> p q", p=128))
        bvb_sb = const.tile([128, 2, DSH], f32)
        nc.sync.dma_start(out=bvb_sb[:, 0], in_=bvb[0])
        nc.sync.dma_start(out=bvb_sb[:, 1], in_=bvb[1])
        modsel = const.tile([128, 2, 6 * KT], f32)   # [p, stream, comp*KT + dt]
        epsb = const.tile([128, 1], f32)
        nc.any.memset(epsb[:], 1e-6)
        onesb = const.tile([128, 64], bf16)
        nc.any.memset(onesb[:], 1.0)
        onesf = const.tile([128, 64], f32)
        nc.any.memset(onesf[:], 1.0)
        g1band = const.tile([128, 2, QT], f32)

        # ---------------- P0: modulation ----------------
        with tc.tile_pool(name="p0", bufs=2) as p0:
            temb_sb = p0.tile([128, KT, 2], f32)
            nc.sync.dma_start(out=temb_sb[:], in_=tembT.rearrange("(kt p) b -> p kt b", p=128))
            stemb = p0.tile([128, KT, 2], f32r)
            nc.scalar.activation(stemb[:], temb_sb[:], AF.Sigmoid)
            nc.any.tensor_tensor(out=stemb[:], in0=stemb[:], in1=temb_sb[:],
                                 op=ALU.mult)

            modb_sb = p0.tile([128, 2, JB], f32)
            nc.sync.dma_start(out=modb_sb[:], in_=modb.rearrange("s j p -> p s j"))
            modag_sb = p0.tile([128, 2, JB, 2], f32)
            for s in range(2):
                for j in range(JB):
                    wtile = p0.tile([128, KT, 128], f32r, tag="modw")
                    nc.sync.dma_start(out=wtile[:], in_=_kp(modw[s])[:, :, ts(j, 128)])
                    ps = ps_aux(128, 2)
                    for kt in range(KT):
                        nc.tensor.matmul(ps[:], wtile[:, kt], stemb[:, kt],
                                         start=(kt == 0), stop=(kt == KT - 1))
                    nc.any.tensor_scalar(out=modag_sb[:, s, j, :], in0=ps[:],
                                         scalar1=modb_sb[:, s, ds(j, 1)],
                                         scalar2=None, op0=ALU.add)
            nc.sync.dma_start(
                out=modag_in.rearrange("(s j p b) -> p s j b", s=2, j=JB, p=128),
                in_=modag_sb[:])
            nc.gpsimd.collective_compute(
                "AllGather", ALU.bypass, replica_groups=ALL8,
                ins=[modag_in[:]], outs=[modag_out[:]])
            # AG layout [c, s, j, p, b]; global block bi = k*KT+dt = j*8+c
            modsb = p0.tile([128, 2, JB, 8, 2], f32)
            for s in range(2):
                for j in range(JB):
                    nc.sync.dma_start(
                        out=modsb[:, s, j],
                        in_=modag_out.rearrange("(c s j p b) -> p s j c b",
                                                c=8, s=2, j=JB, p=128)[:, s, j])
            # collapse batch via bsel one-hot
            modt0 = p0.tile([128, 2, JB, 8], f32)
            nc.any.tensor_scalar(out=modt0[:], in0=modsb[:, :, :, :, 0],
                                 scalar1=bsel_sb[:, 0:1], scalar2=None, op0=ALU.mult)
            modt1 = p0.tile([128, 2, JB, 8], f32)
            nc.any.tensor_scalar(out=modt1[:], in0=modsb[:, :, :, :, 1],
                                 scalar1=bsel_sb[:, 1:2], scalar2=None, op0=ALU.mult)
            nc.any.tensor_tensor(out=modsel[:].rearrange("p s (j c) -> p s j c", j=JB),
                                 in0=modt0[:], in1=modt1[:], op=ALU.add)
            # +1 on the scale components
            for s in range(2):
                for comp in (C_SC1, C_SC2):
                    nc.any.tensor_scalar(out=modsel[:, s, ds(comp * KT, KT)],
                                         in0=modsel[:, s, ds(comp * KT, KT)],
                                         scalar1=1.0, scalar2=None, op0=ALU.add)
            # band g1 scalars: g1band[p, s, mt] = sum_dt g1[p, dt] * tselb[p, mt, dt]
            for s in range(2):
                for mt in range(QT):
                    tmp = p0.tile([128, KT], f32, tag="g1tmp")
                    nc.any.tensor_tensor(out=tmp[:], in0=tselb_sb[:, mt],
                                         in1=modsel[:, s, ds(C_G1 * KT, KT)],
                                         op=ALU.mult)
                    nc.vector.tensor_reduce(g1band[:, s, ds(mt, 1)], tmp[:],
                                            AX.X, ALU.add)

        if DEBUG:
            nc.sync.dma_start(out=d_mod[:], in_=modsel[:])

        def msc(s, comp, dtile):
            return modsel[:, s, ds(comp * KT + dtile, 1)]

        # ---------------- P1: x -> xT (bf16), spill, modulate ----------------
        xtp = tc.alloc_tile_pool(name="xt", bufs=1, side="right")
        xt_sb = xtp.tile([128, KT, S], bf16)
        with tc.tile_pool(name="p1", bufs=2) as p1:
            for tt in range(TT):
                xtok = p1.tile([128, D], f32, tag="xtok")
                nc.sync.dma_start(out=xtok[:], in_=x[ts(tt, 128), :])
                xtokb = p1.tile([128, D], bf16, tag="xtokb")
                nc.gpsimd.tensor_copy(out=xtokb[:], in_=xtok[:])
                for dtile in range(KT):
                    pst = ps_aux(128, 128, bf16)
                    nc.tensor.transpose(pst[:], xtokb[:, ts(dtile, 128)], idb[:])
                    nc.any.tensor_copy(out=xt_sb[:, dtile, ts(tt, 128)], in_=pst[:])
            nc.sync.dma_start(out=xt_dram.rearrange("k p s -> p k s"), in_=xt_sb[:])

        xmodp = tc.alloc_tile_pool(name="xmod", bufs=1)
        xmod = xmodp.tile([128, KT, S], bf16)
        for dtile in range(KT):
            for s, lo, sz in ((0, 0, SI), (1, SI, ST)):
                nc.any.tensor_scalar(out=xmod[:, dtile, ds(lo, sz)],
                                     in0=xt_sb[:, dtile, ds(lo, sz)],
                                     scalar1=msc(s, C_SC1, dtile),
                                     scalar2=msc(s, C_SH1, dtile),
                                     op0=ALU.mult, op1=ALU.add)
        if DEBUG:
            nc.sync.dma_start(out=d_xt[:], in_=xt_sb[:])
            nc.sync.dma_start(out=d_xmod[:], in_=xmod[:])
        xtp.release()

        # ---------------- P2: QKV + rmsnorm factors ----------------
        qkvp = tc.alloc_tile_pool(name="qkv", bufs=1, side="right")
        qT = qkvp.tile([128, QT, S], bf16)
        kT = qkvp.tile([128, QT, S], bf16)
        vaug = qkvp.tile([128, TT, HPC * (HD + 1)], bf16)
        RG = (HPC + 3) // 4   # head groups of <=4, spaced 32 partitions apart
        rqrows = qkvp.tile([128, RG, S], bf16)
        scalek = qkvp.tile([128, TT, HPC], f32)
        nc.any.memset(
            vaug[:].rearrange("p t (h w) -> p t h w", h=HPC)[:, :, :, HD:], 1.0)

        with tc.tile_pool(name="p2w", bufs=2, side="right") as p2w, \
             tc.tile_pool(name="p2", bufs=2, side="right") as p2:
            wqk_b = p2w.tile([128, KT, 2, 2 * DSH], bf16, tag="wqkw", bufs=1,
                             name="wqk_b")
            for kt in range(KT):
                for s in range(2):
                    wq_f = p2w.tile([128, 2 * DSH], f32, tag="wqf", bufs=1)
                    nc.sync.dma_start(out=wq_f[:],
                                      in_=wqkv[ts(kt, 128), ds(s * 3 * DSH, 2 * DSH)])
                    nc.gpsimd.tensor_copy(out=wqk_b[:, kt, s], in_=wq_f[:])

            for ci, (lo, sz) in enumerate(CHUNKS):
                s = 0 if lo < SI else 1
                sqc = [p2.tile([128, QT, 512], bf16, name=f"sq{i}", tag=f"sq{i}",
                               bufs=1) for i in range(2)]
                for part in range(2):     # 0 = Q, 1 = K
                    dst = qT if part == 0 else kT
                    for mt in range(QT):
                        ps = ps_mm()
                        for kt in range(KT):
                            nc.tensor.matmul(ps[:, :sz],
                                             wqk_b[:, kt, s, ds(part * DSH + mt * 128, 128)],
                                             xmod[:, kt, ds(lo, sz)],
                                             start=(kt == 0), stop=(kt == KT - 1))
                        qb = p2.tile([128, 512], f32, tag="qbtmp")
                        nc.any.tensor_scalar(
                            out=qb[:, :sz], in0=ps[:, :sz],
                            scalar1=bqkv_sb[:, ds((s * 3 + part) * QT + mt, 1)],
                            scalar2=None, op0=ALU.add)
                        nc.any.tensor_tensor(out=sqc[part][:, mt, :sz],
                                             in0=qb[:, :sz], in1=qb[:, :sz],
                                             op=ALU.mult)
                        nc.any.tensor_scalar(
                            out=dst[:, mt, ds(lo, sz)], in0=qb[:, :sz],
                            scalar1=nw_sb[:, mt, ds(s * 2 + part, 1)],
                            scalar2=None, op0=ALU.mult)
                # rms factors for this chunk's token tiles
                for tt4 in range(sz // 128):
                    tt = lo // 128 + tt4
                    rqtile = p2.tile([128, RG, 128], f32, tag="rqtile")
                    nc.any.memset(rqtile[:], 0.0)
                    for part in range(2):
                        ps6 = ps_aux(128, HPC)
                        for kq in range(QT):
                            nc.tensor.matmul(ps6[:], sqc[part][:, kq, ts(tt4, 128)],
                                             hmask_sb[:, kq], start=(kq == 0),
                                             stop=(kq == QT - 1))
                        sr = p2.tile([128, HPC], f32, tag="sqrttmp")
                        nc.scalar.activation(sr[:], ps6[:], AF.Sqrt,
                                             bias=epsb[:], scale=1.0 / HD)
                        if part == 1:
                            nc.vector.reciprocal(out=scalek[:, tt], in_=sr[:])
                        else:
                            for g in range(RG):
                                nh = min(4, HPC - 4 * g)
                                nc.vector.reciprocal(
                                    out=rqtile[:, g, 0:32 * nh:32],
                                    in_=sr[:, ds(4 * g, nh)])
                    for g in range(RG):
                        psr = ps_aux(128, 128)
                        nc.tensor.transpose(psr[:], rqtile[:, g], idf[:])
                        nc.any.tensor_copy(out=rqrows[:, g, ts(tt, 128)], in_=psr[:])

            # V pass (token-major); wv tile reuses the qk weight slot
            wv_b = p2w.tile([128, KT, 2, DSH], bf16, tag="wqkw", bufs=1,
                            name="wv_b")
            for kt in range(KT):
                for s in range(2):
                    wq_f = p2w.tile([128, 2 * DSH], f32, tag="wqf", bufs=1)
                    nc.sync.dma_start(out=wq_f[:, :DSH],
                                      in_=wqkv[ts(kt, 128), ds(s * 3 * DSH + 2 * DSH, DSH)])
                    nc.gpsimd.tensor_copy(out=wv_b[:, kt, s], in_=wq_f[:, :DSH])
            for ci, (lo, sz) in enumerate(CHUNKS):
                s = 0 if lo < SI else 1
                for tt4 in range(sz // 128):
                    tt = lo // 128 + tt4
                    ps = ps_mm(DSH) if DSH != 512 else ps_mm()
                    for kt in range(KT):
                        nc.tensor.matmul(ps[:, :DSH], xmod[:, kt, ds(lo + tt4 * 128, 128)],
                                         wv_b[:, kt, s],
                                         start=(kt == 0), stop=(kt == KT - 1))
                    nc.any.tensor_tensor(
                        out=vaug[:, tt].rearrange("p (h w) -> p h w", h=HPC)[:, :, :HD],
                        in0=ps[:, :DSH].rearrange("p (h w) -> p h w", h=HPC),
                        in1=bvb_sb[:, s].rearrange("p (h w) -> p h w", h=HPC),
                        op=ALU.add)

            # fold 1/sqrt(HD) into the k-side exp scale
            nc.any.tensor_scalar(out=scalek[:], in0=scalek[:],
                                 scalar1=float(1.0 / np.sqrt(HD)), scalar2=None,
                                 op0=ALU.mult)
            # q side: replicate each head row over its 64 partitions with a
            # K=1 PE outer product, multiply into qT straight from psum
            for h in range(HPC):
                po = (h % 2) * 64
                bp = 32 * (h % 4)
                for ci, (lo, sz) in enumerate(CHUNKS):
                    psb = psp.tile([64, 512], f32, name="psbq", tag="bcast", bufs=1)
                    nc.tensor.matmul(psb[:, :sz], onesb[ds(bp, 1), :],
                                     rqrows[ds(bp, 1), h // 4, ds(lo, sz)],
                                     start=True, stop=True,
                                     tile_position=(bp, 0) if bp == 96 else None)
                    nc.any.tensor_tensor(out=qT[ds(po, HD), h // 2, ds(lo, sz)],
                                         in0=qT[ds(po, HD), h // 2, ds(lo, sz)],
                                         in1=psb[:HD, :sz], op=ALU.mult)

        if DEBUG:
            nc.sync.dma_start(out=d_qt[:], in_=qT[:])
            nc.sync.dma_start(out=d_kt[:], in_=kT[:])
            nc.sync.dma_start(out=d_scalek[:], in_=scalek[:])
            nc.sync.dma_start(out=d_rqrows[:], in_=rqrows[:])
            nc.sync.dma_start(out=d_vaug[:], in_=vaug[:])
        xmodp.release()

        # ---------------- P3: attention ----------------
        attnp = tc.alloc_tile_pool(name="attn", bufs=1)
        attnT = attnp.tile([128, QT, S], bf16)
        with tc.tile_pool(name="p3", bufs=2, side="right") as p3:
            for ci, (lo, sz) in enumerate(CHUNKS):
                for hp in range(HPC // 2):
                    eab = [p3.tile([128, TT, 512], bf16, name=f"e{i}", tag=f"e{i}", bufs=1)
                           for i in range(2)]
                    for kt in range(TT):
                        for i in range(2):
                            h = 2 * hp + i
                            po = i * 64
                            pss = ps_mm()
                            nc.tensor.matmul(pss[:, :sz],
                                             kT[ds(po, HD), hp, ts(kt, 128)],
                                             qT[ds(po, HD), hp, ds(lo, sz)],
                                             start=True, stop=True)
                            nc.scalar.activation(eab[i][:, kt, :sz], pss[:, :sz],
                                                 AF.Exp, scale=scalek[:, kt, ds(h, 1)])
                    if DEBUG and ci == 0 and hp == 0:
                        nc.sync.dma_start(out=d_e[:], in_=eab[0][:])
                    for i in range(2):
                        h = 2 * hp + i
                        po = i * 64
                        pso = ps_pv()
                        for kt in range(TT):
                            nc.tensor.matmul(pso[:, :sz],
                                             vaug[:, kt, ds(h * (HD + 1), HD + 1)],
                                             eab[i][:, kt, :sz],
                                             start=(kt == 0), stop=(kt == TT - 1))
                        den = p3.tile([1, 512], f32, tag=f"den{i}")
                        nc.any.tensor_copy(out=den[:, :sz], in_=pso[ds(HD, 1), :sz])
                        rec = p3.tile([1, 512], f32, tag=f"rec{i}")
                        nc.vector.reciprocal(out=rec[:, :sz], in_=den[:, :sz])
                        rb = p3.tile([128, 512], f32, tag=f"rb{i}")
                        nc.gpsimd.partition_broadcast(rb[:, :sz], rec[:, :sz])
                        nc.any.tensor_tensor(out=attnT[ds(po, HD), hp, ds(lo, sz)],
                                             in0=pso[ds(0, HD), :sz],
                                             in1=rb[ds(po, HD), :sz], op=ALU.mult)

        qkvp.release()

        # ---------------- P4: AG1, out-proj, AG2 ----------------
        if DEBUG:
            nc.sync.dma_start(out=d_attn[:], in_=attnT[:])
        nc.sync.dma_start(out=ag1_in.rearrange("q p s -> p q s"), in_=attnT[:])
        nc.gpsimd.collective_compute("AllGather", ALU.bypass, replica_groups=GROUPS,
                                     ins=[ag1_in[:]], outs=[ag1_out[:]])
        d1p = tc.alloc_tile_pool(name="d1", bufs=1, side="right")
        d1band = d1p.tile([128, QT, S], f32)
        with tc.tile_pool(name="p4", bufs=2, side="right") as p4:
            wo_b = p4.tile([128, KT, 2, DSH], bf16, bufs=1)
            for kt in range(KT):
                wo_f = p4.tile([128, 2, DSH], f32, tag="wof")
                nc.sync.dma_start(out=wo_f[:],
                                  in_=wout.rearrange("s (k p) m -> p k s m", p=128)[:, kt])
                nc.gpsimd.tensor_copy(out=wo_b[:, kt], in_=wo_f[:])
            d1bf = p4.tile([128, QT, S], bf16, bufs=1)
            for ci, (lo, sz) in enumerate(CHUNKS):
                s = 0 if lo < SI else 1
                rhs = p4.tile([128, KT, 512], bf16, tag="agrhs")
                nc.sync.dma_start(
                    out=rhs[:, :, :sz],
                    in_=ag1_out.rearrange("k p s -> p k s")[:, :, ds(lo, sz)])
                for mt in range(QT):
                    ps = ps_mm()
                    for kt in range(KT):
                        nc.tensor.matmul(ps[:, :sz],
                                         wo_b[:, kt, s, ds(mt * 128, 128)],
                                         rhs[:, kt, :sz], start=(kt == 0),
                                         stop=(kt == KT - 1))
                    # D1 = (y1 + bo) * g1   (band rows)
                    nc.any.tensor_scalar(out=d1band[:, mt, ds(lo, sz)], in0=ps[:, :sz],
                                         scalar1=bout_sb[:, s, ds(mt, 1)],
                                         scalar2=g1band[:, s, ds(mt, 1)],
                                         op0=ALU.add, op1=ALU.mult)
                    nc.any.tensor_copy(out=d1bf[:, mt, ds(lo, sz)],
                                       in_=d1band[:, mt, ds(lo, sz)])
            if DEBUG:
                nc.sync.dma_start(out=d_d1[:], in_=d1band[:])
            nc.sync.dma_start(out=ag2_in.rearrange("q p s -> p q s"), in_=d1bf[:])
        attnp.release()
        nc.gpsimd.collective_compute("AllGather", ALU.bypass, replica_groups=GROUPS,
                                     ins=[ag2_in[:]], outs=[ag2_out[:]])

        # ---------------- P5: FFN ----------------
        with tc.tile_pool(name="p5ff", bufs=1) as p5ffp, \
             tc.tile_pool(name="p5h", bufs=1) as p5hp, \
             tc.tile_pool(name="p5", bufs=2) as p5:
            ff = p5ffp.tile([128, KT, S], bf16)
            for kt in range(KT):
                xtc = p5.tile([128, S], bf16, tag="xtc")
                nc.sync.dma_start(out=xtc[:], in_=xt_dram[kt])
                d1c = p5.tile([128, S], bf16, tag="d1c")
                nc.sync.dma_start(out=d1c[:], in_=ag2_out[kt])
                nc.any.tensor_tensor(out=ff[:, kt, :], in0=xtc[:], in1=d1c[:],
                                     op=ALU.add)
                for s, lo, sz in ((0, 0, SI), (1, SI, ST)):
                    nc.any.tensor_scalar(out=ff[:, kt, ds(lo, sz)],
                                         in0=ff[:, kt, ds(lo, sz)],
                                         scalar1=msc(s, C_SC2, kt),
                                         scalar2=msc(s, C_SH2, kt),
                                         op0=ALU.mult, op1=ALU.add)
            if DEBUG:
                nc.sync.dma_start(out=d_ff[:], in_=ff[:])
            hbuf = p5hp.tile([128, MT, S], bf16)
            for mt in range(MT):
                w1bs = []
                for s in range(2):
                    w1f = p5.tile([128, KT, 128], f32, tag="wstg")
                    nc.sync.dma_start(out=w1f[:], in_=_kp(w1[s])[:, :, ts(mt, 128)])
                    w1b = p5.tile([128, KT, 128], bf16, tag=f"wb{s}")
                    nc.gpsimd.tensor_copy(out=w1b[:], in_=w1f[:])
                    w1bs.append(w1b)
                for ci, (lo, sz) in enumerate(CHUNKS):
                    s = 0 if lo < SI else 1
                    ps = ps_mm()
                    for kt in range(KT):
                        nc.tensor.matmul(ps[:, :sz], w1bs[s][:, kt],
                                         ff[:, kt, ds(lo, sz)],
                                         start=(kt == 0), stop=(kt == KT - 1))
                    if not SIMCOMPOSE:
                        nc.scalar.activation(hbuf[:, mt, ds(lo, sz)], ps[:, :sz],
                                             AF.Gelu_apprx_tanh,
                                             bias=b1_sb[:, s, ds(mt, 1)])
                    else:
                        tmp = p5.tile([128, 512], f32, tag="gelt")
                        nc.any.tensor_scalar(out=tmp[:, :sz], in0=ps[:, :sz],
                                             scalar1=b1_sb[:, s, ds(mt, 1)],
                                             scalar2=None, op0=ALU.add)
                        t2 = p5.tile([128, 512], f32, tag="gelt2")
                        nc.any.tensor_tensor(out=t2[:, :sz], in0=tmp[:, :sz],
                                             in1=tmp[:, :sz], op=ALU.mult)
                        nc.any.tensor_scalar(out=t2[:, :sz], in0=t2[:, :sz],
                                             scalar1=0.044715, scalar2=1.0,
                                             op0=ALU.mult, op1=ALU.add)
                        nc.any.tensor_tensor(out=t2[:, :sz], in0=t2[:, :sz],
                                             in1=tmp[:, :sz], op=ALU.mult)
                        nc.scalar.activation(t2[:, :sz], t2[:, :sz], AF.Tanh,
                                             scale=0.7978845608028654)
                        nc.any.tensor_scalar(out=t2[:, :sz], in0=t2[:, :sz],
                                             scalar1=0.5, scalar2=0.5,
                                             op0=ALU.mult, op1=ALU.add)
                        nc.any.tensor_tensor(out=hbuf[:, mt, ds(lo, sz)],
                                             in0=t2[:, :sz], in1=tmp[:, :sz],
                                             op=ALU.mult)
            for dtile in range(KT):
                w2bs = []
                for s in range(2):
                    w2f = p5.tile([128, MT, 128], f32, tag="wstg")
                    nc.sync.dma_start(out=w2f[:], in_=_kp(w2[s])[:, :, ts(dtile, 128)])
                    w2b = p5.tile([128, MT, 128], bf16, tag=f"wb{s}")
                    nc.gpsimd.tensor_copy(out=w2b[:], in_=w2f[:])
                    w2bs.append(w2b)
                for ci, (lo, sz) in enumerate(CHUNKS):
                    s = 0 if lo < SI else 1
                    ps = ps_mm()
                    for mt in range(MT):
                        nc.tensor.matmul(ps[:, :sz], w2bs[s][:, mt],
                                         hbuf[:, mt, ds(lo, sz)],
                                         start=(mt == 0), stop=(mt == MT - 1))
                    y2 = p5.tile([128, 512], f32, tag="y2e")
                    nc.any.tensor_scalar(out=y2[:, :sz], in0=ps[:, :sz],
                                         scalar1=b2q_sb[:, s, ds(dtile, 1)],
                                         scalar2=msc(s, C_G2, dtile),
                                         op0=ALU.add, op1=ALU.mult)
                    nc.sync.dma_start(out=rs_in[dtile, :, ds(lo, sz)], in_=y2[:, :sz])
            if DEBUG:
                nc.sync.dma_start(out=d_hb[:], in_=hbuf[:])
        nc.gpsimd.collective_compute("ReduceScatter", ALU.add, replica_groups=GROUPS,
                                     ins=[rs_in[:]], outs=[rs_out[:]])

        # ---------------- P6: outputs ----------------
        with tc.tile_pool(name="p6a", bufs=1) as p6a:
            d12 = p6a.tile([128, QT, S], f32)
            nc.sync.dma_start(out=d12[:], in_=rs_out.rearrange("q p s -> p q s"))
            if DEBUG:
                nc.sync.dma_start(out=d_rs[:], in_=rs_out.rearrange("q p s -> p q s"))
            nc.any.tensor_tensor(out=d12[:], in0=d12[:], in1=d1band[:], op=ALU.add)
            d1p.release()
            xb_sb = p6a.tile([128, TT, DSH], f32)
            nc.sync.dma_start(out=xb_sb[:], in_=xband.rearrange("(t p) m -> p t m", p=128))
            hidtok = p6a.tile([128, TT, DSH], f32)
            for mt in range(QT):
                for tt in range(TT):
                    pst = ps_aux()
                    nc.tensor.transpose(pst[:], d12[:, mt, ts(tt, 128)], idf[:])
                    nc.any.tensor_tensor(out=hidtok[:, tt, ts(mt, 128)], in0=pst[:],
                                         in1=xb_sb[:, tt, ts(mt, 128)], op=ALU.add)
            nc.sync.dma_start(out=o_hid.rearrange("(t p) m -> p t m", p=128), in_=hidtok[:])
            # img_h2^T band (feature-major, img tokens) -> AG3
            h2T = p6a.tile([128, QT, SI], f32r)
            for mt in range(QT):
                for tt in range(ITT):
                    pst = ps_aux()
                    nc.tensor.transpose(pst[:], hidtok[:, tt, ts(mt, 128)], idf[:])
                    nc.any.tensor_copy(out=h2T[:, mt, ts(tt, 128)], in_=pst[:])
            nc.sync.dma_start(out=ag3_in.rearrange("q p s -> p q s"), in_=h2T[:])
        nc.gpsimd.collective_compute("AllGather", ALU.bypass, replica_groups=GROUPS,
                                     ins=[ag3_in[:]], outs=[ag3_out[:]])
        with tc.tile_pool(name="p6b", bufs=1) as p6b, \
             tc.tile_pool(name="p6s", bufs=2) as p6s:
            apw_sb = p6b.tile([128, KT, DSH], f32r)
            nc.sync.dma_start(out=apw_sb[:], in_=_kp(apw))
            ctlT = p6b.tile([128, QT, SI], f32)
            for ci, (lo, sz) in enumerate(ICHUNKS):
                rhs = p6s.tile([128, KT, 512], f32r, tag="aprhs")
                nc.sync.dma_start(
                    out=rhs[:, :, :sz],
                    in_=ag3_out.rearrange("k p s -> p k s")[:, :, ds(lo, sz)])
                for mt in range(QT):
                    ps = ps_mm()
                    for kt in range(KT):
                        nc.tensor.matmul(ps[:, :sz], apw_sb[:, kt, ts(mt, 128)],
                                         rhs[:, kt, :sz], start=(kt == 0),
                                         stop=(kt == KT - 1))
                    nc.any.tensor_scalar(out=ctlT[:, mt, ds(lo, sz)], in0=ps[:, :sz],
                                         scalar1=apb_sb[:, ds(mt, 1)], scalar2=None,
                                         op0=ALU.add)
            ctok = p6b.tile([128, ITT, DSH], f32)
            for mt in range(QT):
                for tt in range(ITT):
                    pst = ps_aux()
                    nc.tensor.transpose(pst[:], ctlT[:, mt, ts(tt, 128)], idf[:])
                    nc.any.tensor_copy(out=ctok[:, tt, ts(mt, 128)], in_=pst[:])
            nc.sync.dma_start(out=o_ctrl.rearrange("(t p) m -> p t m", p=128), in_=ctok[:])

        const.release()
        psp.release()

    nc.finalize()
    return nc


# ====================== host side ======================

def _shard_inputs(inputs):
    npf = np.float32
    ident = np.eye(128, dtype=npf)
    identb = np.eye(128, dtype=ml_dtypes.bfloat16)
    g = lambda n: np.asarray(inputs[n], npf)
    in_maps = []
    for c in range(NCORES):
        b, t = c // TP, c % TP
        cs = slice(DSH * t, DSH * (t + 1))
        ms = slice(MSH * t, MSH * (t + 1))
        xfull = np.concatenate([g("img_hidden")[b], g("txt_hidden")[b]], 0)
        modw = np.stack([
            g("img_mod_w").reshape(D, MODB, 128)[:, c::8, :].reshape(D, JB * 128),
            g("txt_mod_w").reshape(D, MODB, 128)[:, c::8, :].reshape(D, JB * 128)])
        modb_ = np.stack([g("img_mod_b").reshape(MODB, 128)[c::8],
                          g("txt_mod_b").reshape(MODB, 128)[c::8]])
        bsel = np.zeros((128, 2), npf)
        bsel[:, b] = 1.0
        tselb = np.zeros((128, QT, KT), npf)
        for mt in range(QT):
            tselb[:, mt, t * QT + mt] = 1.0
        wqkv = np.concatenate([g("wq")[:, cs], g("wk")[:, cs], g("wv")[:, cs],
                               g("awq")[:, cs], g("awk")[:, cs], g("awv")[:, cs]], 1)
        bqkv = np.concatenate([g("bq")[cs], g("bk")[cs], g("bv")[cs],
                               g("abq")[cs], g("abk")[cs], g("abv")[cs]])
        nw = np.stack([np.tile(g("nq_w"), HPC), np.tile(g("nk_w"), HPC),
                       np.tile(g("naq_w"), HPC), np.tile(g("nak_w"), HPC)], 1)
        hmask = np.zeros((DSH, HPC), ml_dtypes.bfloat16)
        for h in range(HPC):
            hmask[h * HD:(h + 1) * HD, h] = 1.0
        bvb = np.stack([np.tile(g("bv")[cs], (128, 1)),
                        np.tile(g("abv")[cs], (128, 1))])
        in_maps.append({
            "x": np.ascontiguousarray(xfull),
            "xband": np.ascontiguousarray(xfull[:, cs]),
            "tembT": np.ascontiguousarray(g("temb").T),
            "modw": np.ascontiguousarray(modw),
            "modb": np.ascontiguousarray(modb_),
            "bsel": bsel, "tselb": tselb,
            "wqkv": np.ascontiguousarray(wqkv), "bqkv": bqkv,
            "nw": np.ascontiguousarray(nw), "hmask": hmask,
            "wout": np.ascontiguousarray(np.stack([g("wo")[:, cs], g("awo")[:, cs]])),
            "bout": np.ascontiguousarray(np.stack([g("bo")[cs], g("abo")[cs]])),
            "w1": np.ascontiguousarray(np.stack([g("img_w1")[:, ms], g("txt_w1")[:, ms]])),
            "b1": np.ascontiguousarray(np.stack([g("img_b1")[ms], g("txt_b1")[ms]])),
            "w2": np.ascontiguousarray(np.stack([g("img_w2")[ms, :], g("txt_w2")[ms, :]])),
            "b2q": np.ascontiguousarray(np.stack([g("img_b2"), g("txt_b2")]) / TP),
            "apw": np.ascontiguousarray(g("ap_w")[:, cs]),
            "apb": np.ascontiguousarray(g("ap_b")[cs]),
            "id128f": ident, "id128b": identb,
            "bvb": np.ascontiguousarray(bvb),
        })
    return in_maps


def _gather_outputs(results):
    img_h = np.zeros((B, SI, D), np.float32)
    txt_h = np.zeros((B, ST, D), np.float32)
    ctrl = np.zeros((B, SI, D), np.float32)
    for c in range(NCORES):
        b, t = c // TP, c % TP
        cs = slice(DSH * t, DSH * (t + 1))
        hid = results[c]["o_hid"]
        img_h[b][:, cs] = hid[:SI]
        txt_h[b][:, cs] = hid[SI:]
        ctrl[b][:, cs] = results[c]["o_ctrl"]
    return img_h, txt_h, ctrl


_NC_CACHE = None


def _get_nc():
    global _NC_CACHE
    if _NC_CACHE is None:
        _NC_CACHE = build()
    return _NC_CACHE


def kernel(**inputs):
    from concourse.bass_utils import run_bass_kernel_spmd
    nc = _get_nc()
    in_maps = _shard_inputs(inputs)
    res = run_bass_kernel_spmd(nc, in_maps, list(range(NCORES)))
    return _gather_outputs(res.results)


def run_sim(inputs):
    from concourse.bass_interp import MultiCoreSim
    nc = _get_nc()
    in_maps = _shard_inputs(inputs)
    sim = MultiCoreSim(nc, NCORES)
    for c in range(NCORES):
        for k, v in in_maps[c].items():
            sim.cores[c].tensor(k)[:] = v
    sim.simulate()
    results = [{k: np.asarray(sim.cores[c].tensor(k)) for k in ("o_hid", "o_ctrl")}
               for c in range(NCORES)]
    return _gather_outputs(results)
